# revision 6
# baseline (speedup 1.0000x reference)
"""Trainium2 Bass kernel for the minGRU encoder (nn_Encoder_65635690218112).

Strategy
--------
- Data-parallel over batch: 16 batches -> 8 cores x 2 batches each; weights
  replicated. Everything stays feature-major (h^T layout), transpose-free.
- h lives in SBUF in bf16 ([128, NJ, S] tiles) for half of T at a time
  (G=2 groups); each layer's weights stream from DRAM once per (group,
  layer) double-buffered, so weight DMA is 4x smaller than reloading per
  (batch, group) and fully hidden.
- Mixed precision split by sensitivity:
  * The update-gate path runs in fp8(e4m3) with perf_mode=DoubleRow (K=256
    per matmul via 3D access patterns over two 128-row slabs). Gate noise
    is damped ~30x by the sigmoid (|zh_z| stays under ~0.7), so fp8 there
    is accuracy-free. A per-layer power-of-two-scaled fp8 shadow copy of h
    feeds it; scales are folded into the activation scale/bias.
  * The candidate path, which feeds h directly, runs in bf16 at the same
    per-column PE rate but full precision. End-to-end rel err ~9e-3
    (vs 6e-3 for all-bf16, 8e-2 for all-fp8).
- Elementwise work is spread across engines so the PE never waits:
  z = sigmoid(psz*s + bz) and h~ = psc + bc on ScalarE (draining PSUM),
  a = 1-z, b = z*h~, the recurrence scan h_t = a_t*h_{t-1} + b_t
  (tensor_tensor_scan, f32 internal carry, bf16 in/out), and the fp8
  shadow downcast all on VectorE. Keeping the shadow downcast on VectorE
  (right behind its producing scan) avoids ScalarE FIFO head-of-line
  blocking, which otherwise stalls the PE a chunk at a time.
- The post projection runs in bf16 straight from h, skewed one chunk
  behind the layer-5 matmuls so the PE never waits on a scan.
"""

import numpy as np
import ml_dtypes

import concourse.bass as bass
import concourse.mybir as mybir
import concourse.tile as tile

# ---------------------------------------------------------------------------
# Workaround: this walrus build accepts at most ONE sem wait per instruction
# ("Too many sync wait commands"). After Tile assigns waits, split any
# instruction carrying more by inserting same-engine NoOps ahead of it.
# ---------------------------------------------------------------------------
from concourse.vector_clock import ScopedClock

_MAX_WAITS = 1
_noop_ctr = [0]


def _split_waits_in_block(bb):
    new_list = []
    for inst in bb.instructions:
        si = getattr(inst, "sync_info", None)
        if si is not None and si.on_wait and len(si.on_wait) > _MAX_WAITS:
            waits = list(si.on_wait)
            keep = waits[-_MAX_WAITS:]
            extra = waits[:-_MAX_WAITS]
            for i in range(0, len(extra), _MAX_WAITS):
                _noop_ctr[0] += 1
                nop = mybir.InstNoOp(
                    name=f"I-waitsplit-{_noop_ctr[0]}",
                    engine=inst.engine,
                    bass_nofuse=True,
                    sync_info=mybir.SyncInfo(
                        on_wait=extra[i : i + _MAX_WAITS], on_update=[]
                    ),
                )
                new_list.append(nop)
            inst.sync_info = mybir.SyncInfo(on_wait=keep, on_update=si.on_update)
        new_list.append(inst)
    bb.instructions[:] = new_list


def _patched_drain_and_barrier(self, tick_clock, wait_clock):
    nc = self.nc
    drain_inst = nc.sync.drain()
    wait_clock.add_sem_waits(
        drain_inst.ins, ScopedClock({None: tick_clock.global_clock})
    )
    for bb in nc.main_func.blocks:
        _split_waits_in_block(bb)
    nc.all_engine_barrier()
    assert self.sems is not None
    popped = nc._tile_sem_poison_stack.pop()
    assert popped is self._sem_poison
    nc.clear_and_free_semaphores(list(self.sems.allocated().values()))
    nc.all_engine_barrier()


tile.TileContext._drain_and_barrier = _patched_drain_and_barrier

# ---------------------------------------------------------------------------

f32 = mybir.dt.float32
bf16 = mybir.dt.bfloat16
fp8 = mybir.dt.float8e4
AF = mybir.ActivationFunctionType
ALU = mybir.AluOpType
PM = mybir.MatmulPerfMode

N_CORES = 8
C_IN = 80
C_OUT = 194
D = 1024
L = 6
T = 4096
S = 512
NJ = D // 128          # 8 feature blocks of 128
KP = NJ // 2           # 4 DoubleRow contraction pair-blocks of 256
BS = 2                 # batches per core
G = 2                  # time groups resident in SBUF
TG = T // G
NCH = TG // S          # chunks per (batch, group)

# fp8 quantization scales (powers of two; e4m3 saturates at 240).
# Weights are iid N(0, 0.02^2); h maxima decay from ~1.1 to ~0.04 by layer.
WSC = 256.0
SH = [32.0, 64.0, 128.0, 256.0, 512.0, 1024.0]  # h-shadow scale entering layer l


def build_program(REP=1):
    nc = bass.Bass()

    x_d = nc.declare_dram_parameter("x", [BS, C_IN, T], bf16, isOutput=False)
    wpre_d = nc.declare_dram_parameter("wpre", [C_IN, D], bf16, isOutput=False)
    bpre_d = nc.declare_dram_parameter("bpre", [128, NJ], f32, isOutput=False)
    # update-gate weights, DoubleRowSwInterleave layout (scaled by WSC, fp8):
    # per (kp, j) a contiguous 256-col window, pairs interleaved + reversed,
    # so the PE weight load streams contiguously (plain DoubleRow's
    # non-contiguous interleave fetch serializes ~140ns/MM extra).
    wz_d = nc.declare_dram_parameter("wz", [L, KP, 128, NJ, 256], fp8,
                                     isOutput=False)
    # candidate weights, bf16 col-tiles
    wc_d = nc.declare_dram_parameter("wc", [L, NJ, 128, D], bf16, isOutput=False)
    # biases: [L, 2, 128, NJ] = (bz, bc)
    bl_d = nc.declare_dram_parameter("bl", [L, 2, 128, NJ], f32, isOutput=False)
    wpost_d = nc.declare_dram_parameter("wpost", [128, NJ * C_OUT], bf16,
                                        isOutput=False)
    bpost_d = nc.declare_dram_parameter("bpost", [128, 2], f32, isOutput=False)
    out_d = nc.declare_dram_parameter("out", [BS, C_OUT, T], f32, isOutput=True)

    INVZ = [1.0 / (WSC * SH[l]) for l in range(L)]

    with tile.TileContext(nc) as tc:
        with (
            tc.tile_pool(name="const", bufs=1) as cpool,
            tc.tile_pool(name="h", bufs=1) as hpool,
            tc.tile_pool(name="w", bufs=1) as wpool,
            tc.tile_pool(name="bias", bufs=1) as bpool,
            tc.tile_pool(name="scr", bufs=1) as spool,
            tc.tile_pool(name="ps", bufs=1, space="PSUM") as pspool,
        ):
            wpre_sb = cpool.tile([C_IN, D], bf16, tag="wpre")
            nc.sync.dma_start(wpre_sb[:], wpre_d[:])
            bpre_sb = cpool.tile([128, NJ], f32, tag="bpre")
            nc.sync.dma_start(bpre_sb[:], bpre_d[:])
            wpost_sb = cpool.tile([128, NJ * C_OUT], bf16, tag="wpost")
            nc.sync.dma_start(wpost_sb[:], wpost_d[:])
            bpost_sb = cpool.tile([128, 2], f32, tag="bpost")
            nc.sync.dma_start(bpost_sb[:], bpost_d[:])
            # scan carries across groups: column (b*L + l)*NJ + j
            carry_sb = cpool.tile([128, BS * L * NJ], bf16, tag="carry")

            # h for one group: [b][c] tiles of [128, NJ, S] (bf16 + fp8 shadow)
            h = [
                [hpool.tile([128, NJ, S], bf16, tag=f"h{b}_{c}", name=f"h{b}_{c}")
                 for c in range(NCH)]
                for b in range(BS)
            ]
            h8 = [
                [hpool.tile([128, NJ, S], fp8, tag=f"h8{b}_{c}", name=f"h8{b}_{c}")
                 for c in range(NCH)]
                for b in range(BS)
            ]

            for _rep in range(REP):
              for g in range(G):
                t0 = g * TG
                # ---- pre-projection: h0 = x^T W_pre + b_pre ----
                for b in range(BS):
                    x_sb = spool.tile([C_IN, TG], bf16, tag="x", bufs=2,
                                      name="x_sb")
                    nc.sync.dma_start(x_sb[:], x_d[b][:, t0 : t0 + TG])
                    for c in range(NCH):
                        for j in range(NJ):
                            ps = pspool.tile(
                                [128, S], f32,
                                tag=("psz" if j % 2 == 0 else "psc"), bufs=4,
                                name="ps_pre",
                            )
                            nc.tensor.matmul(
                                ps[:],
                                wpre_sb[:, j * 128 : (j + 1) * 128],
                                x_sb[:, c * S : (c + 1) * S],
                                start=True,
                                stop=True,
                            )
                            nc.scalar.activation(
                                h[b][c][:, j, :], ps[:], AF.Identity,
                                bias=bpre_sb[:, j : j + 1], scale=1.0,
                            )
                            nc.vector.tensor_scalar(
                                h8[b][c][:, j, :], h[b][c][:, j, :],
                                SH[0], None, op0=ALU.mult,
                            )

                # ---- the L minGRU layers ----
                prev_meta = None
                for i in range(L):
                    last = i == L - 1
                    wz = []
                    for kp in range(KP):
                        wt = wpool.tile([128, NJ, 256], fp8, tag=f"wz{kp}",
                                        bufs=2, name=f"wz{kp}")
                        nc.sync.dma_start(wt[:], wz_d[i, kp])
                        wz.append(wt)
                    wc = []
                    for kb in range(NJ):
                        wt = wpool.tile([128, D], bf16, tag=f"wc{kb}",
                                        bufs=2, name=f"wc{kb}")
                        nc.sync.dma_start(wt[:], wc_d[i, kb])
                        wc.append(wt)
                    bz = bpool.tile([128, NJ], f32, tag="bz", bufs=2, name="bz")
                    nc.sync.dma_start(bz[:], bl_d[i, 0])
                    bc = bpool.tile([128, NJ], f32, tag="bc", bufs=2, name="bc")
                    nc.sync.dma_start(bc[:], bl_d[i, 1])

                    for b in range(BS):
                        for c in range(NCH):
                            a_ts, b_ts = [], []
                            for j in range(NJ):
                                psz = pspool.tile(
                                    [128, S], f32, tag="psz", bufs=4, name="psz"
                                )
                                psc = pspool.tile(
                                    [128, S], f32, tag="psc", bufs=4, name="psc"
                                )
                                for kp in range(KP):
                                    nc.tensor.matmul(
                                        psz[:],
                                        wz[kp][:, j, :],
                                        h8[b][c][:, 2 * kp : 2 * kp + 2, :],
                                        start=(kp == 0),
                                        stop=(kp == KP - 1),
                                        perf_mode=PM.DoubleRowSwInterleave,
                                    )
                                for kb in range(NJ):
                                    nc.tensor.matmul(
                                        psc[:],
                                        wc[kb][:, j * 128 : (j + 1) * 128],
                                        h[b][c][:, kb, :],
                                        start=(kb == 0),
                                        stop=(kb == NJ - 1),
                                    )
                                z_t = spool.tile(
                                    [128, S], bf16, tag="z", bufs=4, name="z_t"
                                )
                                ht_t = spool.tile(
                                    [128, S], bf16, tag="ht", bufs=4, name="ht_t"
                                )
                                a_t = spool.tile(
                                    [128, S], bf16, tag="a", bufs=8, name="a_t"
                                )
                                b_t = spool.tile(
                                    [128, S], bf16, tag="bb", bufs=8, name="b_t"
                                )
                                nc.scalar.activation(
                                    z_t[:], psz[:], AF.Sigmoid,
                                    bias=bz[:, j : j + 1], scale=INVZ[i],
                                )
                                nc.scalar.activation(
                                    ht_t[:], psc[:], AF.Identity,
                                    bias=bc[:, j : j + 1], scale=1.0,
                                )
                                nc.vector.tensor_scalar(
                                    a_t[:], z_t[:], -1.0, 1.0,
                                    op0=ALU.mult, op1=ALU.add,
                                )
                                nc.vector.tensor_tensor(
                                    b_t[:], z_t[:], ht_t[:], op=ALU.mult
                                )
                                a_ts.append(a_t)
                                b_ts.append(b_t)

                            # ---- scans (after all matmuls of this chunk) ----
                            for j in range(NJ):
                                if g == 0 and c == 0:
                                    init = 0.0
                                elif c == 0:
                                    ci = (b * L + i) * NJ + j
                                    init = carry_sb[:, ci : ci + 1]
                                else:
                                    init = h[b][c - 1][:, j, S - 1 : S]
                                nc.vector.tensor_tensor_scan(
                                    h[b][c][:, j, :], a_ts[j][:], b_ts[j][:],
                                    init, op0=ALU.mult, op1=ALU.add,
                                )
                                if not last:
                                    nc.vector.tensor_scalar(
                                        h8[b][c][:, j, :], h[b][c][:, j, :],
                                        SH[i + 1], None, op0=ALU.mult,
                                    )
                            if g == 0 and c == NCH - 1:
                                for j in range(NJ):
                                    ci = (b * L + i) * NJ + j
                                    nc.vector.tensor_copy(
                                        carry_sb[:, ci : ci + 1],
                                        h[b][c][:, j, S - 1 : S],
                                    )

                            if last:
                                if prev_meta is not None:
                                    _emit_post(nc, pspool, spool, wpost_sb,
                                               bpost_sb, out_d, h, t0, prev_meta)
                                prev_meta = (b, c)
                    if last and prev_meta is not None:
                        _emit_post(nc, pspool, spool, wpost_sb, bpost_sb,
                                   out_d, h, t0, prev_meta)
                        prev_meta = None
    return nc


def _emit_post(nc, pspool, spool, wpost_sb, bpost_sb, out_d, h, t0, meta):
    b, c = meta
    for p, (p0, pw) in enumerate(((0, 128), (128, C_OUT - 128))):
        ps_o = pspool.tile(
            [128, S], f32, tag=("psz" if p == 0 else "psc"), bufs=4,
            name="ps_o",
        )
        for kb in range(NJ):
            nc.tensor.matmul(
                ps_o[:pw, :],
                wpost_sb[:, kb * C_OUT + p0 : kb * C_OUT + p0 + pw],
                h[b][c][:, kb, :],
                start=(kb == 0),
                stop=(kb == NJ - 1),
            )
        o_t = spool.tile([128, S], f32, tag="o", bufs=4, name="o_t")
        nc.scalar.activation(
            o_t[:pw, :], ps_o[:pw, :], AF.Identity,
            bias=bpost_sb[:pw, p : p + 1], scale=1.0,
        )
        nc.sync.dma_start(
            out_d[b][p0 : p0 + pw, t0 + c * S : t0 + (c + 1) * S],
            o_t[:pw, :],
        )


def pack_inputs(x, w_pre, b_pre, w_layers, b_layers, w_post, b_post):
    """Host-side packing: bf16 x/pre/candidate/post, fp8 gate weights."""
    e4 = ml_dtypes.float8_e4m3
    bfl = ml_dtypes.bfloat16

    x = np.ascontiguousarray(np.asarray(x, np.float32).astype(bfl))
    w_pre = np.ascontiguousarray(np.asarray(w_pre, np.float32).astype(bfl))
    bpre = np.ascontiguousarray(
        np.asarray(b_pre, np.float32).reshape(NJ, 128).T
    )

    wlf = np.asarray(w_layers, np.float32)            # [L, D, 2D]
    # gate SwInterleave layout: per (kp, j) window of 256, pairs interleaved
    # and columns reversed: s[p, j, 2u+k] = W[(2kp+k)*128+p, j*128+127-u]*WSC
    wz = wlf[:, :, :D].reshape(L, KP, 2, 128, NJ, 128)   # [l,kp,k,p,j,m]
    wz = wz.transpose(0, 1, 3, 4, 5, 2)                  # [l,kp,p,j,m,k]
    wz = wz[:, :, :, :, ::-1, :]                         # m -> u = 127-m
    wz = np.ascontiguousarray(
        (wz * WSC).astype(e4).reshape(L, KP, 128, NJ, 256))
    # candidate col tiles: wc[l, kb, p, m] = W[l, kb*128+p, D+m]
    wc = wlf[:, :, D:].reshape(L, NJ, 128, D)
    wc = np.ascontiguousarray(wc.astype(bfl))

    blf = np.asarray(b_layers, np.float32)
    bl = np.empty((L, 2, 128, NJ), np.float32)
    bl[:, 0] = blf[:, :D].reshape(L, NJ, 128).transpose(0, 2, 1)
    bl[:, 1] = blf[:, D:].reshape(L, NJ, 128).transpose(0, 2, 1)
    bl = np.ascontiguousarray(bl)

    wpost = (
        np.asarray(w_post, np.float32)
        .reshape(NJ, 128, C_OUT)
        .transpose(1, 0, 2)
        .reshape(128, NJ * C_OUT)
    )
    wpost = np.ascontiguousarray(wpost.astype(bfl))
    bpost = np.zeros((128, 2), np.float32)
    bpost[:, 0] = np.asarray(b_post[:128], np.float32)
    bpost[: C_OUT - 128, 1] = np.asarray(b_post[128:], np.float32)
    return x, w_pre, bpre, wz, wc, bl, wpost, bpost


PACK_NAMES = ["x", "wpre", "bpre", "wz", "wc", "bl", "wpost", "bpost"]

_program_cache = {}


def _get_program(key):
    if key not in _program_cache:
        REP = key[0]
        _program_cache[key] = build_program(REP=REP)
    return _program_cache[key]


def run(inputs, REP=1, trace=False):
    from concourse.bass_utils import run_bass_kernel_spmd

    packed = pack_inputs(
        inputs["x"], inputs["w_pre"], inputs["b_pre"], inputs["w_layers"],
        inputs["b_layers"], inputs["w_post"], inputs["b_post"],
    )
    x = packed[0]
    shared = dict(zip(PACK_NAMES[1:], packed[1:]))
    nc = _get_program((REP,))
    in_maps = [
        {"x": np.ascontiguousarray(x[c * BS : (c + 1) * BS]), **shared}
        for c in range(N_CORES)
    ]
    res = run_bass_kernel_spmd(nc, in_maps, list(range(N_CORES)), trace=trace)
    out = np.concatenate([res.results[c]["out"] for c in range(N_CORES)], axis=0)
    return out, res


def kernel(**inputs):
    out, _ = run(inputs)
    return out


# revision 7
# speedup vs baseline: 1.1111x; 1.1111x over previous
"""Trainium2 Bass kernel for the minGRU encoder (nn_Encoder_65635690218112).

Strategy
--------
- Data-parallel over batch: 16 batches -> 8 cores x 2 batches each; weights
  replicated. Everything stays feature-major (h^T layout), transpose-free.
- h lives in SBUF in bf16 ([128, NJ, S] tiles) for half of T at a time
  (G=2 groups); each layer's weights stream from DRAM once per (group,
  layer) double-buffered, so weight DMA is 4x smaller than reloading per
  (batch, group) and fully hidden.
- Mixed precision split by sensitivity:
  * The update-gate path runs in fp8(e4m3) with perf_mode=DoubleRow (K=256
    per matmul via 3D access patterns over two 128-row slabs). Gate noise
    is damped ~30x by the sigmoid (|zh_z| stays under ~0.7), so fp8 there
    is accuracy-free. A per-layer power-of-two-scaled fp8 shadow copy of h
    feeds it; scales are folded into the activation scale/bias.
  * The candidate path, which feeds h directly, runs in bf16 at the same
    per-column PE rate but full precision. End-to-end rel err ~9e-3
    (vs 6e-3 for all-bf16, 8e-2 for all-fp8).
- Elementwise work is spread across engines so the PE never waits:
  z = sigmoid(psz*s + bz) and h~ = psc + bc on ScalarE (draining PSUM),
  a = 1-z, b = z*h~, the recurrence scan h_t = a_t*h_{t-1} + b_t
  (tensor_tensor_scan, f32 internal carry, bf16 in/out), and the fp8
  shadow downcast all on VectorE. Keeping the shadow downcast on VectorE
  (right behind its producing scan) avoids ScalarE FIFO head-of-line
  blocking, which otherwise stalls the PE a chunk at a time.
- The post projection runs in bf16 straight from h, skewed one chunk
  behind the layer-5 matmuls so the PE never waits on a scan.
"""

import numpy as np
import ml_dtypes

import concourse.bass as bass
import concourse.mybir as mybir
import concourse.tile as tile

# ---------------------------------------------------------------------------
# Workaround: this walrus build accepts at most ONE sem wait per instruction
# ("Too many sync wait commands"). After Tile assigns waits, split any
# instruction carrying more by inserting same-engine NoOps ahead of it.
# ---------------------------------------------------------------------------
from concourse.vector_clock import ScopedClock

_MAX_WAITS = 1
_noop_ctr = [0]


def _split_waits_in_block(bb):
    new_list = []
    for inst in bb.instructions:
        si = getattr(inst, "sync_info", None)
        if si is not None and si.on_wait and len(si.on_wait) > _MAX_WAITS:
            waits = list(si.on_wait)
            keep = waits[-_MAX_WAITS:]
            extra = waits[:-_MAX_WAITS]
            for i in range(0, len(extra), _MAX_WAITS):
                _noop_ctr[0] += 1
                nop = mybir.InstNoOp(
                    name=f"I-waitsplit-{_noop_ctr[0]}",
                    engine=inst.engine,
                    bass_nofuse=True,
                    sync_info=mybir.SyncInfo(
                        on_wait=extra[i : i + _MAX_WAITS], on_update=[]
                    ),
                )
                new_list.append(nop)
            inst.sync_info = mybir.SyncInfo(on_wait=keep, on_update=si.on_update)
        new_list.append(inst)
    bb.instructions[:] = new_list


def _patched_drain_and_barrier(self, tick_clock, wait_clock):
    nc = self.nc
    drain_inst = nc.sync.drain()
    wait_clock.add_sem_waits(
        drain_inst.ins, ScopedClock({None: tick_clock.global_clock})
    )
    for bb in nc.main_func.blocks:
        _split_waits_in_block(bb)
    nc.all_engine_barrier()
    assert self.sems is not None
    popped = nc._tile_sem_poison_stack.pop()
    assert popped is self._sem_poison
    nc.clear_and_free_semaphores(list(self.sems.allocated().values()))
    nc.all_engine_barrier()


tile.TileContext._drain_and_barrier = _patched_drain_and_barrier

# ---------------------------------------------------------------------------

f32 = mybir.dt.float32
bf16 = mybir.dt.bfloat16
fp8 = mybir.dt.float8e4
AF = mybir.ActivationFunctionType
ALU = mybir.AluOpType
PM = mybir.MatmulPerfMode

N_CORES = 8
C_IN = 80
C_OUT = 194
D = 1024
L = 6
T = 4096
S = 512
NJ = D // 128          # 8 feature blocks of 128
KP = NJ // 2           # 4 DoubleRow contraction pair-blocks of 256
BS = 2                 # batches per core
G = 2                  # time groups resident in SBUF
TG = T // G
NCH = TG // S          # chunks per (batch, group)

# fp8 quantization scales (powers of two; e4m3 saturates at 240).
# Weights are iid N(0, 0.02^2); h maxima decay from ~1.1 to ~0.04 by layer.
WSC = 256.0
SH = [32.0, 64.0, 128.0, 256.0, 512.0, 1024.0]  # h-shadow scale entering layer l


def build_program(REP=1):
    nc = bass.Bass()

    x_d = nc.declare_dram_parameter("x", [BS, C_IN, T], bf16, isOutput=False)
    wpre_d = nc.declare_dram_parameter("wpre", [C_IN, D], bf16, isOutput=False)
    bpre_d = nc.declare_dram_parameter("bpre", [128, NJ], f32, isOutput=False)
    # update-gate weights, DoubleRow layout (scaled by WSC, fp8)
    wz_d = nc.declare_dram_parameter("wz", [L, KP, 128, 2, D], fp8, isOutput=False)
    # candidate weights, bf16 col-tiles
    wc_d = nc.declare_dram_parameter("wc", [L, NJ, 128, D], bf16, isOutput=False)
    # biases: [L, 2, 128, NJ] = (bz, bc)
    bl_d = nc.declare_dram_parameter("bl", [L, 2, 128, NJ], f32, isOutput=False)
    wpost_d = nc.declare_dram_parameter("wpost", [128, NJ * C_OUT], bf16,
                                        isOutput=False)
    bpost_d = nc.declare_dram_parameter("bpost", [128, 2], f32, isOutput=False)
    out_d = nc.declare_dram_parameter("out", [BS, C_OUT, T], f32, isOutput=True)

    INVZ = [1.0 / (WSC * SH[l]) for l in range(L)]

    with tile.TileContext(nc) as tc:
        with (
            tc.tile_pool(name="const", bufs=1) as cpool,
            tc.tile_pool(name="h", bufs=1) as hpool,
            tc.tile_pool(name="w", bufs=1) as wpool,
            tc.tile_pool(name="bias", bufs=1) as bpool,
            tc.tile_pool(name="scr", bufs=1) as spool,
            tc.tile_pool(name="ps", bufs=1, space="PSUM") as pspool,
        ):
            wpre_sb = cpool.tile([C_IN, D], bf16, tag="wpre")
            nc.sync.dma_start(wpre_sb[:], wpre_d[:])
            bpre_sb = cpool.tile([128, NJ], f32, tag="bpre")
            nc.sync.dma_start(bpre_sb[:], bpre_d[:])
            wpost_sb = cpool.tile([128, NJ * C_OUT], bf16, tag="wpost")
            nc.sync.dma_start(wpost_sb[:], wpost_d[:])
            bpost_sb = cpool.tile([128, 2], f32, tag="bpost")
            nc.sync.dma_start(bpost_sb[:], bpost_d[:])
            # scan carries across groups: column (b*L + l)*NJ + j
            carry_sb = cpool.tile([128, BS * L * NJ], bf16, tag="carry")

            # h for one group: [b][c] tiles of [128, NJ, S] (bf16 + fp8 shadow)
            h = [
                [hpool.tile([128, NJ, S], bf16, tag=f"h{b}_{c}", name=f"h{b}_{c}")
                 for c in range(NCH)]
                for b in range(BS)
            ]
            h8 = [
                [hpool.tile([128, NJ, S], fp8, tag=f"h8{b}_{c}", name=f"h8{b}_{c}")
                 for c in range(NCH)]
                for b in range(BS)
            ]

            for _rep in range(REP):
              for g in range(G):
                t0 = g * TG
                # ---- pre-projection: h0 = x^T W_pre + b_pre ----
                for b in range(BS):
                    x_sb = spool.tile([C_IN, TG], bf16, tag="x", bufs=2,
                                      name="x_sb")
                    nc.sync.dma_start(x_sb[:], x_d[b][:, t0 : t0 + TG])
                    for c in range(NCH):
                        for j in range(NJ):
                            ps = pspool.tile(
                                [128, S], f32,
                                tag=("psz" if j % 2 == 0 else "psc"), bufs=4,
                                name="ps_pre",
                            )
                            nc.tensor.matmul(
                                ps[:],
                                wpre_sb[:, j * 128 : (j + 1) * 128],
                                x_sb[:, c * S : (c + 1) * S],
                                start=True,
                                stop=True,
                            )
                            nc.scalar.activation(
                                h[b][c][:, j, :], ps[:], AF.Identity,
                                bias=bpre_sb[:, j : j + 1], scale=1.0,
                            )
                            nc.vector.tensor_scalar(
                                h8[b][c][:, j, :], h[b][c][:, j, :],
                                SH[0], None, op0=ALU.mult,
                            )

                # ---- the L minGRU layers ----
                prev_meta = None
                for i in range(L):
                    last = i == L - 1
                    wz = []
                    for kp in range(KP):
                        wt = wpool.tile([128, 2, D], fp8, tag=f"wz{kp}",
                                        bufs=2, name=f"wz{kp}")
                        nc.sync.dma_start(wt[:], wz_d[i, kp])
                        wz.append(wt)
                    wc = []
                    for kb in range(NJ):
                        wt = wpool.tile([128, D], bf16, tag=f"wc{kb}",
                                        bufs=2, name=f"wc{kb}")
                        nc.sync.dma_start(wt[:], wc_d[i, kb])
                        wc.append(wt)
                    bz = bpool.tile([128, NJ], f32, tag="bz", bufs=2, name="bz")
                    nc.sync.dma_start(bz[:], bl_d[i, 0])
                    bc = bpool.tile([128, NJ], f32, tag="bc", bufs=2, name="bc")
                    nc.sync.dma_start(bc[:], bl_d[i, 1])

                    for b in range(BS):
                        for c in range(NCH):
                            a_ts, b_ts = [], []
                            for j in range(NJ):
                                psz = pspool.tile(
                                    [128, S], f32, tag="psz", bufs=4, name="psz"
                                )
                                psc = pspool.tile(
                                    [128, S], f32, tag="psc", bufs=4, name="psc"
                                )
                                for kp in range(KP):
                                    nc.tensor.matmul(
                                        psz[:],
                                        wz[kp][:, :, j * 128 : (j + 1) * 128],
                                        h8[b][c][:, 2 * kp : 2 * kp + 2, :],
                                        start=(kp == 0),
                                        stop=(kp == KP - 1),
                                        perf_mode=PM.DoubleRow,
                                    )
                                for kb in range(NJ):
                                    nc.tensor.matmul(
                                        psc[:],
                                        wc[kb][:, j * 128 : (j + 1) * 128],
                                        h[b][c][:, kb, :],
                                        start=(kb == 0),
                                        stop=(kb == NJ - 1),
                                    )
                                z_t = spool.tile(
                                    [128, S], bf16, tag="z", bufs=4, name="z_t"
                                )
                                ht_t = spool.tile(
                                    [128, S], bf16, tag="ht", bufs=4, name="ht_t"
                                )
                                a_t = spool.tile(
                                    [128, S], bf16, tag="a", bufs=8, name="a_t"
                                )
                                b_t = spool.tile(
                                    [128, S], bf16, tag="bb", bufs=8, name="b_t"
                                )
                                nc.scalar.activation(
                                    z_t[:], psz[:], AF.Sigmoid,
                                    bias=bz[:, j : j + 1], scale=INVZ[i],
                                )
                                nc.scalar.activation(
                                    ht_t[:], psc[:], AF.Identity,
                                    bias=bc[:, j : j + 1], scale=1.0,
                                )
                                nc.vector.tensor_scalar(
                                    a_t[:], z_t[:], -1.0, 1.0,
                                    op0=ALU.mult, op1=ALU.add,
                                )
                                nc.vector.tensor_tensor(
                                    b_t[:], z_t[:], ht_t[:], op=ALU.mult
                                )
                                a_ts.append(a_t)
                                b_ts.append(b_t)

                            # ---- scans (after all matmuls of this chunk) ----
                            for j in range(NJ):
                                if g == 0 and c == 0:
                                    init = 0.0
                                elif c == 0:
                                    ci = (b * L + i) * NJ + j
                                    init = carry_sb[:, ci : ci + 1]
                                else:
                                    init = h[b][c - 1][:, j, S - 1 : S]
                                nc.vector.tensor_tensor_scan(
                                    h[b][c][:, j, :], a_ts[j][:], b_ts[j][:],
                                    init, op0=ALU.mult, op1=ALU.add,
                                )
                                if not last:
                                    nc.vector.tensor_scalar(
                                        h8[b][c][:, j, :], h[b][c][:, j, :],
                                        SH[i + 1], None, op0=ALU.mult,
                                    )
                            if g == 0 and c == NCH - 1:
                                for j in range(NJ):
                                    ci = (b * L + i) * NJ + j
                                    nc.vector.tensor_copy(
                                        carry_sb[:, ci : ci + 1],
                                        h[b][c][:, j, S - 1 : S],
                                    )

                            if last:
                                if prev_meta is not None:
                                    _emit_post(nc, pspool, spool, wpost_sb,
                                               bpost_sb, out_d, h, t0, prev_meta)
                                prev_meta = (b, c)
                    if last and prev_meta is not None:
                        _emit_post(nc, pspool, spool, wpost_sb, bpost_sb,
                                   out_d, h, t0, prev_meta)
                        prev_meta = None
    return nc


def _emit_post(nc, pspool, spool, wpost_sb, bpost_sb, out_d, h, t0, meta):
    b, c = meta
    for p, (p0, pw) in enumerate(((0, 128), (128, C_OUT - 128))):
        ps_o = pspool.tile(
            [128, S], f32, tag=("psz" if p == 0 else "psc"), bufs=4,
            name="ps_o",
        )
        for kb in range(NJ):
            nc.tensor.matmul(
                ps_o[:pw, :],
                wpost_sb[:, kb * C_OUT + p0 : kb * C_OUT + p0 + pw],
                h[b][c][:, kb, :],
                start=(kb == 0),
                stop=(kb == NJ - 1),
            )
        o_t = spool.tile([128, S], f32, tag="o", bufs=4, name="o_t")
        nc.scalar.activation(
            o_t[:pw, :], ps_o[:pw, :], AF.Identity,
            bias=bpost_sb[:pw, p : p + 1], scale=1.0,
        )
        nc.sync.dma_start(
            out_d[b][p0 : p0 + pw, t0 + c * S : t0 + (c + 1) * S],
            o_t[:pw, :],
        )


def pack_inputs(x, w_pre, b_pre, w_layers, b_layers, w_post, b_post):
    """Host-side packing: bf16 x/pre/candidate/post, fp8 gate weights."""
    e4 = ml_dtypes.float8_e4m3
    bfl = ml_dtypes.bfloat16

    x = np.ascontiguousarray(np.asarray(x, np.float32).astype(bfl))
    w_pre = np.ascontiguousarray(np.asarray(w_pre, np.float32).astype(bfl))
    bpre = np.ascontiguousarray(
        np.asarray(b_pre, np.float32).reshape(NJ, 128).T
    )

    wlf = np.asarray(w_layers, np.float32)            # [L, D, 2D]
    # gate DoubleRow layout: wz[l, kp, p, k, m] = W[l, (2kp+k)*128+p, m] * WSC
    wz = wlf[:, :, :D].reshape(L, KP, 2, 128, D).transpose(0, 1, 3, 2, 4)
    wz = np.ascontiguousarray((wz * WSC).astype(e4))
    # candidate col tiles: wc[l, kb, p, m] = W[l, kb*128+p, D+m]
    wc = wlf[:, :, D:].reshape(L, NJ, 128, D)
    wc = np.ascontiguousarray(wc.astype(bfl))

    blf = np.asarray(b_layers, np.float32)
    bl = np.empty((L, 2, 128, NJ), np.float32)
    bl[:, 0] = blf[:, :D].reshape(L, NJ, 128).transpose(0, 2, 1)
    bl[:, 1] = blf[:, D:].reshape(L, NJ, 128).transpose(0, 2, 1)
    bl = np.ascontiguousarray(bl)

    wpost = (
        np.asarray(w_post, np.float32)
        .reshape(NJ, 128, C_OUT)
        .transpose(1, 0, 2)
        .reshape(128, NJ * C_OUT)
    )
    wpost = np.ascontiguousarray(wpost.astype(bfl))
    bpost = np.zeros((128, 2), np.float32)
    bpost[:, 0] = np.asarray(b_post[:128], np.float32)
    bpost[: C_OUT - 128, 1] = np.asarray(b_post[128:], np.float32)
    return x, w_pre, bpre, wz, wc, bl, wpost, bpost


PACK_NAMES = ["x", "wpre", "bpre", "wz", "wc", "bl", "wpost", "bpost"]

_program_cache = {}


def _get_program(key):
    if key not in _program_cache:
        REP = key[0]
        _program_cache[key] = build_program(REP=REP)
    return _program_cache[key]


def run(inputs, REP=1, trace=False):
    from concourse.bass_utils import run_bass_kernel_spmd

    packed = pack_inputs(
        inputs["x"], inputs["w_pre"], inputs["b_pre"], inputs["w_layers"],
        inputs["b_layers"], inputs["w_post"], inputs["b_post"],
    )
    x = packed[0]
    shared = dict(zip(PACK_NAMES[1:], packed[1:]))
    nc = _get_program((REP,))
    in_maps = [
        {"x": np.ascontiguousarray(x[c * BS : (c + 1) * BS]), **shared}
        for c in range(N_CORES)
    ]
    res = run_bass_kernel_spmd(nc, in_maps, list(range(N_CORES)), trace=trace)
    out = np.concatenate([res.results[c]["out"] for c in range(N_CORES)], axis=0)
    return out, res


def kernel(**inputs):
    out, _ = run(inputs)
    return out
